# revision 1
# baseline (speedup 1.0000x reference)
"""Damerau-Levenshtein nn_Module kernel for Trainium2 (Bass/Tile), 8-core SPMD.

Sharding: the 10000-word dictionary is split across 8 cores (1250 words/core,
padded to 1280 = 128 partitions x 2 passes x 5 chunks); each core runs the
full DP for all 32 (b,s) query elements against its shard and the host
concatenates per-core outputs (pure data-parallel, no collectives).

Algorithm: the reference's Damerau-Levenshtein DP (Lowrance-Wagner da/db
variant, with the reference's exact border quirks) is reformulated into
per-row wide vector ops:
  - the serial inner column loop becomes one min-plus tensor_tensor_scan
    (insertion chain);
  - the transposition gather d[k][l] becomes, per past row k, a "hold" scan
    (state = notmatch*state + match*inject) merged over k by copy_predicated
    with precomputed masks (B_j == A_k); ascending k gives last-occurrence
    (da) semantics, the hold structure gives the db column semantics, and
    the scan masks are shifted one column so the state read excludes the
    current cell's own match;
  - all DP values are stored "tilted": DP row r is kept as d[r][col] - col
    (and history rows with a further -(r+2)), folding every per-column iota
    and per-k constant into the data, so the hot loop is mult/min/predicate;
  - rows beyond swl+1 never affect the output, so (b,s) elements are sorted
    by swl descending and each DP row processes only the still-active prefix
    of the free axis;
  - hold-scans run on GPSIMD while v-builds/merges run on DVE (software
    pipelined with double-buffered value/scan tiles); the char-compare mask
    bank (row-independent) is precomputed per pass on GPSIMD.
DP values are small integers, so bf16 is exact; scans accumulate in fp32.
Access patterns are partition + <=2 free dims (neuronxcc limit); broadcast
operands are shipped pre-replicated from the host (bf16 / int8).
"""
import numpy as np

BSZ, SEQ = 2, 16
NBS = BSZ * SEQ            # 32
W = 10000
ML = 14                    # max_len
L2 = ML + 2                # 16 (segment width)
NCORES = 8
WPC = W // NCORES          # 1250
WPAD = 1280                # 128 * 10
NCHUNK = 10
CHP = 5                    # chunks per pass
NPASS = 2
P = 128
SEG = CHP * L2             # 80
FW = NBS * SEG             # 2560
BIG = 1.0e9
# bf16 packed input sections (element offsets)
OFF_DT0 = 0                              # maxdist - col - 2
OFF_DT1 = OFF_DT0 + NPASS * FW           # row1 - col - 3
OFF_MD = OFF_DT1 + NPASS * FW            # maxdist
OFF_ZEROS = OFF_MD + NPASS * NBS * CHP   # scan pattern col0=BIG else 0
OFF_KVAL = OFF_ZEROS + L2                # kvals[r-2] = r+2 for hist tilt
OFF_BPAD = OFF_KVAL + ML                 # B chars at cols 2..15, else -1
OFF_AREP = OFF_BPAD + NPASS * FW         # A chars
TOTF = OFF_AREP + (ML + 1) * NBS * CHP
# int8 packed input sections
OFF_WSEL = 0
TOTI = OFF_WSEL + NPASS * FW


def _build_nc(cnt):
    """cnt[i] = #bs-elements with swl >= i (active prefix for DP row i)."""
    import concourse.bass as bass
    import concourse.mybir as mybir
    from concourse import tile

    cnt = [int(c) for c in cnt]
    op = mybir.AluOpType
    f32 = mybir.dt.float32
    bf16 = mybir.dt.bfloat16
    u8 = mybir.dt.uint8
    i8 = mybir.dt.int8
    AP = bass.AP

    # HW allows at most 1 semaphore wait on CTRL-class (drain) instructions;
    # split the stock end-of-schedule global drain into a chain.
    if not getattr(tile.TileContext, "_dl_drain_patched", False):
        def _split_drain_and_barrier(self, tick_clock, wait_clock):
            from concourse.vector_clock import ScopedClock
            drain_inst = self.nc.sync.drain()
            wait_clock.add_sem_waits(
                drain_inst.ins, ScopedClock({None: tick_clock.global_clock})
            )
            si = drain_inst.ins.sync_info
            if si is not None and si.on_wait is not None and len(si.on_wait) > 1:
                waits = list(si.on_wait)
                si.on_wait = waits[:1]
                drain_inst.ins.sync_info = si
                rest = waits[1:]
                while rest:
                    d2 = self.nc.sync.drain()
                    si2 = d2.ins.sync_info
                    if si2 is None:
                        si2 = mybir.SyncInfo(on_wait=[], on_update=[])
                    si2.on_wait = rest[:1]
                    d2.ins.sync_info = si2
                    rest = rest[1:]
            self.nc.all_engine_barrier()
            assert self.sems is not None
            popped = self.nc._tile_sem_poison_stack.pop()
            assert popped is self._sem_poison
            self.nc.clear_and_free_semaphores(list(self.sems.allocated().values()))
            self.nc.all_engine_barrier()

        tile.TileContext._drain_and_barrier = _split_drain_and_barrier
        tile.TileContext._dl_drain_patched = True

    nc = bass.Bass()
    # register const APs for the ACT biases used below (-16..-2), mirroring
    # Bass.__init__'s register_const_ap closure
    for _val in range(-16, -1):
        _t = nc.alloc_sbuf_tensor(f"const-float32-{float(_val)}", [128, 1], f32)
        nc.gpsimd.memset(_t.ap(), float(_val))
        nc.const_aps.aps[(f32, float(_val))] = _t.ap()
    nc.all_engine_barrier()
    inpf_in = nc.declare_dram_parameter("inpf", [P, TOTF], bf16, isOutput=False)
    inpi_in = nc.declare_dram_parameter("inpi", [P, TOTI], i8, isOutput=False)
    lev_out = nc.declare_dram_parameter("levout", [P, NPASS * NBS * CHP], f32, isOutput=True)

    with tile.TileContext(nc) as tc:
        _frees = []

        def mktile(shape, dtp, name):
            t, f = tc.tile(shape, dtp, name=name)
            _frees.append(f)
            return t

        t_inf = mktile([P, TOTF], bf16, "t_inf")
        t_ini = mktile([P, TOTI], i8, "t_ini")
        bank = mktile([P, ML, NBS, CHP, L2], i8, "bank")   # slot k-1 for k=1..14
        zerosm = mktile([P, NBS, CHP, L2], bf16, "zerosm")
        rowA = mktile([P, NBS, CHP, L2], bf16, "rowA")
        rowB = mktile([P, NBS, CHP, L2], bf16, "rowB")
        nmsh = mktile([P, NBS, CHP, L2], bf16, "nmsh")
        vbuf0 = mktile([P, NBS, CHP, L2], bf16, "vbuf0")
        vbuf1 = mktile([P, NBS, CHP, L2], bf16, "vbuf1")
        glbuf0 = mktile([P, NBS, CHP, L2], bf16, "glbuf0")
        glbuf1 = mktile([P, NBS, CHP, L2], bf16, "glbuf1")
        accb = mktile([P, NBS, CHP, L2], bf16, "accb")
        baseb = mktile([P, NBS, CHP, L2], bf16, "baseb")
        hist = mktile([P, 12, NBS, CHP, L2], bf16, "hist")
        levb = mktile([P, NPASS, NBS, CHP], f32, "levb")
        nc._tile_keepalive = _frees

        nc.sync.dma_start(out=t_inf, in_=inpf_in[:, :])
        nc.sync.dma_start(out=t_ini, in_=inpi_in[:, :])

        v = nc.vector
        g = nc.gpsimd

        def hT(ap):
            return ap.tensor

        def wk3(t, nbs, c0=0, c1=L2):
            return AP(hT(t), c0, [[FW, P], [L2, nbs * CHP], [1, c1 - c0]])

        def inf3(off, ps, nbs, c0=0, c1=L2, elem0=0):
            return AP(hT(t_inf), off + ps * FW + elem0 * SEG + c0,
                      [[TOTF, P], [L2, nbs * CHP], [1, c1 - c0]])

        def ini3(off, ps, nbs, c0=0, c1=L2, elem0=0):
            return AP(hT(t_ini), off + ps * FW + elem0 * SEG + c0,
                      [[TOTI, P], [L2, nbs * CHP], [1, c1 - c0]])

        def arep3(idx, nbs, ncols):
            return AP(hT(t_inf), OFF_AREP + idx * NBS * CHP,
                      [[TOTF, P], [1, nbs * CHP], [0, ncols]])

        def md3(ps, nbs):
            return AP(hT(t_inf), OFF_MD + ps * NBS * CHP,
                      [[TOTF, P], [1, nbs * CHP], [0, 1]])

        def zer3(nbs, c0=0, c1=L2):
            return AP(hT(t_inf), OFF_ZEROS + c0, [[TOTF, P], [0, nbs * CHP], [1, c1 - c0]])

        def flat2d(t, nbs):
            return AP(hT(t), 0, [[FW, P], [1, nbs * SEG]])

        def bank3(k, nbs, c0=0, c1=L2):
            return AP(hT(bank), (k - 1) * FW + c0,
                      [[ML * FW, P], [L2, nbs * CHP], [1, c1 - c0]])

        def hist3(k, nbs, c0=0, c1=L2):
            return AP(hT(hist), (k - 2) * FW + c0,
                      [[12 * FW, P], [L2, nbs * CHP], [1, c1 - c0]])

        # one-time init
        v.tensor_copy(wk3(zerosm, NBS), zer3(NBS))
        v.memset(AP(hT(levb), 0, [[NPASS * NBS * CHP, P], [1, NPASS * NBS * CHP]]), 0.0)
        g.memset(AP(hT(bank), 0, [[ML * FW, P], [L2, ML * NBS * CHP], [1, 2]]), 0)
        g.memset(wk3(nmsh, NBS, 0, 1), 0.0)
        g.memset(wk3(nmsh, NBS, 1, 2), 1.0)
        v.memset(wk3(vbuf0, NBS, 1, 2), 0.0)
        v.memset(wk3(vbuf1, NBS, 1, 2), 0.0)

        for ps in range(NPASS):
            for k0 in (1, 2):
                if cnt[k0] > 0:
                    v.tensor_tensor(bank3(k0, cnt[k0], 2), inf3(OFF_BPAD, ps, cnt[k0], 2),
                                    arep3(k0 - 1, cnt[k0], L2 - 2), op.is_equal)

            # prev row 1 (tilted -col) = dt1' + 3
            v.tensor_scalar_add(wk3(rowA, NBS), inf3(OFF_DT1, ps, NBS), 3.0)

            prev = rowA
            deferred = []
            for i in range(1, ML + 1):
                nbs = cnt[i]
                if nbs == 0:
                    break
                cur = rowB if (i % 2 == 1) else rowA

                # nmsh for this row = 1 - bank_i (shifted one col), on ACT
                nc.scalar.activation(wk3(nmsh, nbs, 2), bank3(i, nbs, 1, L2 - 1),
                                     mybir.ActivationFunctionType.Copy,
                                     bias=1.0, scale=-1.0)
                # GP: precompute bank slot i+2 (slots 1,2 done at pass start)
                if i + 2 <= ML and cnt[i + 2] > 0:
                    kk = i + 2
                    v.tensor_tensor(bank3(kk, cnt[kk], 2), inf3(OFF_BPAD, ps, cnt[kk], 2),
                                    arep3(kk - 1, cnt[kk], L2 - 2), op.is_equal)

                # ACT: deletion t1 candidate (baseb); t2/base-min deferred past
                # the k-loop so next-row scans aren't FIFO-blocked on them
                nc.scalar.add(wk3(baseb, nbs), wk3(prev, nbs), 1.0)

                # k loop, software pipelined: v-build (GP) -> scan (DVE) -> pred (DVE)
                pending_pred = None
                for k in range(0, i):
                    vb = vbuf0 if (k % 2 == 0) else vbuf1
                    nc.scalar.add(wk3(vb, nbs, 0, 1), md3(ps, nbs), -(float(k) + 2.0))
                    if k == 0:
                        dsrc = inf3(OFF_DT0, ps, nbs, 0, L2 - 2)
                    elif k == 1:
                        dsrc = inf3(OFF_DT1, ps, nbs, 0, L2 - 2)
                    else:
                        dsrc = hist3(k, nbs, 0, L2 - 2)
                    g.tensor_tensor(wk3(vb, nbs, 2), dsrc, bank3(i, nbs, 1, L2 - 1), op.mult)
                    dst = accb if k == 0 else (glbuf0 if (k % 2 == 0) else glbuf1)
                    v.tensor_tensor_scan(flat2d(dst, nbs), flat2d(nmsh, nbs), flat2d(vb, nbs),
                                         0.0, op.mult, op.add)
                    if pending_pred is not None:
                        pk, pdst = pending_pred
                        v.copy_predicated(wk3(accb, nbs, 2), bank3(pk, nbs, 2),
                                          wk3(pdst, nbs, 2))
                    pending_pred = (k, dst) if k > 0 else None
                    if k == 1 and deferred:
                        for fn in deferred:
                            fn()
                        deferred = []
                if pending_pred is not None:
                    pk, pdst = pending_pred
                    v.copy_predicated(wk3(accb, nbs, 2), bank3(pk, nbs, 2), wk3(pdst, nbs, 2))

                if deferred:
                    for fn in deferred:
                        fn()
                    deferred = []

                # substitution candidate t2 = prev_shift - bank_i, folded into base
                g.tensor_tensor(wk3(glbuf1, nbs, 1), wk3(prev, nbs, 0, L2 - 1),
                                bank3(i, nbs, 1), op.subtract)
                v.tensor_tensor(wk3(baseb, nbs, 1), wk3(baseb, nbs, 1),
                                wk3(glbuf1, nbs, 1), op.min)

                # trv = acc + i; base = min(base, trv)
                v.tensor_scalar_add(wk3(accb, nbs, 1), wk3(accb, nbs, 1), float(i))
                v.tensor_tensor(wk3(baseb, nbs, 1), wk3(baseb, nbs, 1),
                                wk3(accb, nbs, 1), op.min)
                # border: col1 value; col0 is dead (zerosm col1 also resets)
                v.memset(wk3(baseb, nbs, 1, 2), float(i) - 1.0)
                # running-min scan (tilted insertion chain) -> cur
                v.tensor_tensor_scan(flat2d(cur, nbs), flat2d(zerosm, nbs), flat2d(baseb, nbs),
                                     BIG, op.add, op.min)
                # extraction for bs with swl == i (tilted; host adds wl+1)
                lo = cnt[i + 1] if i < ML else 0
                if nbs > lo:
                    nsl = nbs - lo
                    cur_sl = AP(hT(cur), lo * SEG, [[FW, P], [L2, nsl * CHP], [1, L2]])
                    ext_sl = AP(hT(vbuf1), lo * SEG, [[FW, P], [L2, nsl * CHP], [1, L2]])
                    red_out = AP(hT(levb), ps * NBS * CHP + lo * CHP,
                                 [[NPASS * NBS * CHP, P], [1, nsl * CHP]])
                    wsl_ap = ini3(OFF_WSEL, ps, nsl, 0, L2, elem0=lo)

                    def _ext(cur_sl=cur_sl, ext_sl=ext_sl, red_out=red_out, wsl_ap=wsl_ap):
                        g.tensor_tensor(ext_sl, cur_sl, wsl_ap, op.mult)
                        v.tensor_reduce(red_out, ext_sl, mybir.AxisListType.X, op.add)
                    deferred.append(_ext)
                # history: hist slot r=i+1 holds cur - (r+2); not read until row i+2
                if i + 1 <= 13 and cnt[i + 1] > 0:
                    nb2 = cnt[i + 1]
                    hdst = hist3(i + 1, nb2)
                    csrc = wk3(cur, nb2)
                    hbias = -(float(i + 1) + 2.0)

                    def _hist(hdst=hdst, csrc=csrc, hbias=hbias):
                        nc.scalar.add(hdst, csrc, hbias)
                    deferred.append(_hist)
                prev = cur

            for fn in deferred:
                fn()
            deferred = []

        nc.sync.dma_start(out=lev_out[:, :], in_=levb)

    # HW wait-slot limits per engine encoding: Pool/SP take 1 sem wait per
    # instruction, DVE-class 2. Split excess waits onto drain carriers
    # inserted immediately before the overloaded instruction. (Skipped under
    # CoreSim, whose instrumentation rejects raw-inserted instructions.)
    import os as _os
    if _os.environ.get("DL_NO_WAITSPLIT"):
        return nc
    LIM = {"Pool": 1, "SP": 1, "DVE": 1, "Activation": 1, "PE": 1}
    nsp = 0
    for f in nc.m.functions:
        for bb in f.blocks:
            lst = bb.instructions
            i = 0
            while i < len(lst):
                ins = lst[i]
                si = ins.sync_info
                nw = len(si.on_wait) if si is not None and si.on_wait else 0
                lim = LIM.get(ins.engine.name, 1)
                if nw > lim:
                    waits = list(si.on_wait)
                    si.on_wait = waits[:lim]
                    ins.sync_info = si
                    rest = waits[lim:]
                    while rest:
                        take, rest = rest[:lim], rest[lim:]
                        d = mybir.InstDrain(name=f"XSPLIT-{nsp}", engine=ins.engine)
                        nsp += 1
                        d.sync_info = mybir.SyncInfo(on_wait=take, on_update=[])
                        lst.insert(i, d)
                        i += 1
                i += 1

    return nc


def kernel(x, words, word_lengths, num_chars):
    import os
    import ml_dtypes
    from concourse.bass_utils import run_bass_kernel_spmd

    bf = ml_dtypes.bfloat16
    x = np.asarray(x)
    words = np.asarray(words)
    wl = np.asarray(word_lengths)

    swl = np.argmax(x, axis=-1).reshape(NBS).astype(np.int64)
    A = x.reshape(NBS, ML + 1)

    order = np.argsort(-swl, kind="stable")
    swl_s = swl[order]
    A_s = A[order]
    cnt = [0] * (ML + 2)
    for i in range(1, ML + 1):
        cnt[i] = int((swl_s >= i).sum())

    nc = _build_nc(cnt)

    col = np.arange(L2, dtype=np.float32)
    zeros = np.zeros((P, L2), np.float32)
    zeros[:, 0] = BIG
    zeros[:, 1] = BIG
    arep = np.broadcast_to(A_s.T.astype(np.float32)[None, :, :, None],
                           (P, ML + 1, NBS, CHP)).astype(bf)

    in_maps = []
    for core in range(NCORES):
        wsl = slice(core * WPC, (core + 1) * WPC)
        wl_pad = np.ones((WPAD,), np.int64)
        wl_pad[:WPC] = wl[wsl]
        wd_pad = np.zeros((WPAD, ML), np.int64)
        wd_pad[:WPC] = words[wsl]

        wd_l = wd_pad.reshape(NPASS, CHP, P, ML).transpose(2, 0, 1, 3)   # (P,NPASS,CHP,ML)
        wl_l = wl_pad.reshape(NPASS, CHP, P).transpose(2, 0, 1)          # (P,NPASS,CHP)

        bpad_c = np.full((P, NPASS, CHP, L2), -1.0, np.float32)
        bpad_c[:, :, :, 2:] = wd_l.astype(np.float32)
        bpad = np.broadcast_to(bpad_c[:, :, None, :, :], (P, NPASS, NBS, CHP, L2)).astype(bf)

        md = (swl_s[None, None, :, None] + wl_l[:, :, None, :]).astype(np.float32)
        dt0 = (md[:, :, :, :, None] - col[None, None, None, None, :] - 2.0).astype(bf)

        ar_ = np.arange(ML + 1, dtype=np.float32)
        wla = np.where(ar_[None, None, None, :] <= wl_l[:, :, :, None],
                       ar_[None, None, None, :], 0.0)
        row1 = np.zeros((P, NPASS, CHP, L2), np.float32)
        row1[:, :, :, 1:] = wla
        dt1_c = row1 - col[None, None, None, :] - 3.0
        dt1 = np.broadcast_to(dt1_c[:, :, None, :, :], (P, NPASS, NBS, CHP, L2)).astype(bf)

        wsel_c = (col[None, None, None, :] == (wl_l[:, :, :, None] + 1.0)).astype(np.int8)
        wsel = np.broadcast_to(wsel_c[:, :, None, :, :], (P, NPASS, NBS, CHP, L2))

        kvals = np.broadcast_to((np.arange(ML, dtype=np.float32) + 4.0), (P, ML))
        inpf = np.concatenate([
            np.ascontiguousarray(dt0).reshape(P, NPASS * FW),
            np.ascontiguousarray(dt1).reshape(P, NPASS * FW),
            np.ascontiguousarray(md.astype(bf)).reshape(P, NPASS * NBS * CHP),
            zeros.astype(bf),
            kvals.astype(bf),
            np.ascontiguousarray(bpad).reshape(P, NPASS * FW),
            np.ascontiguousarray(arep).reshape(P, (ML + 1) * NBS * CHP),
        ], axis=1)
        inpi = np.ascontiguousarray(wsel).reshape(P, NPASS * FW)
        in_maps.append({"inpf": inpf, "inpi": inpi})

    trace = bool(os.environ.get("DL_KERNEL_TRACE"))
    res = run_bass_kernel_spmd(nc, in_maps, list(range(NCORES)), trace=trace)
    if trace:
        print("HW exec time:", res.exec_time_ns, "ns", flush=True)
        kernel.last_result = res
    if os.environ.get("DL_KERNEL_BENCH"):
        import time as _time
        walls = []
        for _ in range(int(os.environ.get("DL_KERNEL_BENCH"))):
            _t0 = _time.time()
            run_bass_kernel_spmd(nc, in_maps, list(range(NCORES)))
            walls.append(_time.time() - _t0)
        print("bench walls (s):", [f"{w:.3f}" for w in walls], flush=True)

    lev_s = np.zeros((NBS, W), np.float32)
    for core in range(NCORES):
        r = res.results[core]["levout"].reshape(P, NPASS, NBS, CHP)
        r = r.transpose(2, 1, 3, 0).reshape(NBS, WPAD)[:, :WPC]
        lev_s[:, core * WPC:(core + 1) * WPC] = r

    # un-tilt: device returned d[swl+1][wl+1] - (wl+1)
    lev_s = lev_s + (wl.astype(np.float32)[None, :] + 1.0)

    zmask = swl_s == 0
    if zmask.any():
        lev_s[zmask, :] = wl.astype(np.float32)[None, :]

    lev = np.empty_like(lev_s)
    lev[order] = lev_s
    return lev.reshape(BSZ, SEQ, W).astype(np.float32)

